# revision 20
# baseline (speedup 1.0000x reference)
"""ColBERT late-interaction kernel for 8 Trainium2 NeuronCores (v2).

Math (per reference):
  x = h @ W + b                      (projection, H=768 -> D=128)
  v = x / ||x||_2(seq axis)          (normalize over the SEQUENCE axis,
                                      norm includes masked tokens)
  sim[q,p,n,l] = <q_v[q,n], p_v[p,l]>  (masked tokens excluded from max)
  scores[q,p] = sum_n max_{l valid} sim[q,p,n,l]
  out = concat(pos_scores, neg_scores, axis=1)   # [96, 192]

Sharding: passage-parallel. Every core projects ALL queries and a 1/8 shard
of pos+neg passages (12+12 batches), computes the full-query x local-passage
score block [96, 24]; the host stitches columns.

v2 design notes:
  - Hidden tensors ship as bf16 (halves HBM traffic); projections contract
    bf16 x bf16 with fp32 PSUM accumulate.
  - No mask tensor at all: the host solves W^T h* = -b (on the bf16-rounded
    W) and substitutes h* for pad slots, so those columns project to ~0 and
    drop out of both the max (true max > 0) and the norm. Invalid tokens are
    moved to a compact "correction" block so the sequence-axis sum-of-squares
    still includes them, exactly as the reference does.
  - Passage batches are sorted by valid count; each tile of 6 batches is
    compacted to W_t columns (tile max valid count, rounded up to 8).
  - Sum-of-squares: one ACT Square per chunk (bias folded) to SBUF, then a
    segmented vector reduce_sum -- avoids the per-batch ACT-accumulate
    instruction-overhead wall.
  - Normalization: one scalar_tensor_tensor per chunk on Vector reads the
    projection PSUM directly: out = (x + b) * rsqrt(ss), bf16 out.
  - MaxSim drain is split across engines. PSUM can only be read by Vector
    (0.96 elem/ns/lane) and Scalar (1.2 elem/ns/lane), and Vector's reduce
    is locked at 1x. Role 'V' blocks: direct vector reduce_max from PSUM.
    Role 'G' blocks: Scalar ACT-copies the sim block to SBUF as bf16,
    GpSimd does the first max-halving level, Vector finishes with 2x-mode
    bf16 tensor_tensor max levels + a short reduce.
  - The sum-over-n runs as a ones-matmul per row-group that ACCUMULATES into
    a single PSUM bank across all 27 groups (start only on the first), so the
    epilogue is one copy + one DMA.
"""

import numpy as np

B, NQ, LP, H, D = 96, 35, 180, 768, 128
NCORES = 8
PB = B // NCORES          # 12 passage batches per core per side
LOCAL_P = 2 * PB          # 24 local passage batches (pos then neg)
QCOLS = B * NQ            # 3360 query columns
KCH = H // 128            # 6 contraction chunks
QCHUNK = 420              # 12 query batches per projection chunk
NQCH = QCOLS // QCHUNK    # 8
NGROUPS = (QCOLS + 127) // 128       # 27 interaction row-groups
BPT = 6                   # passage batches per tile
NTILES = LOCAL_P // BPT   # 4
NCORR = 3                 # correction chunks
CORR_B = LOCAL_P // NCORR            # 8 batches per correction chunk


# Groups g < N_DIRECT drain tile 0 by a direct vector reduce_max from PSUM
# and tiles 1-3 through the scalar-copy + vector TT-max tree; groups
# g >= N_DIRECT push all 4 tiles through the tree (cheaper for Vector,
# pricier for Scalar) — the knob balances the two engines.
N_DIRECT = 16


def _build(tile_w, imax):
    import concourse.bacc as bacc
    from concourse import mybir
    from concourse.tile import TileContext

    f32 = mybir.dt.float32
    bf16 = mybir.dt.bfloat16

    tile_w = list(tile_w)
    # half-tile layout: per tile, 2 PSUM banks x 3 batches x W columns
    assert all(3 * w <= 512 for w in tile_w)
    pmain = 6 * sum(tile_w)               # compacted passage columns
    pcorr = LOCAL_P * imax                # correction columns
    # flat per-partition layouts (chunk-major inside each chunk)
    p_offs = []                           # (dram_off, xpn_off, ncols) per half-tile
    off = 0
    xoff = 0
    for t in range(NTILES):
        for h in range(2):
            p_offs.append((off, xoff, 3 * tile_w[t]))
            off += KCH * 3 * tile_w[t]
            xoff += 3 * tile_w[t]
    c_offs = []
    for c in range(NCORR):
        c_offs.append((off, CORR_B * imax))
        off += KCH * CORR_B * imax

    nc = bacc.Bacc(target_bir_lowering=False)

    QH = nc.dram_tensor("qh", [NQCH, 128, KCH * QCHUNK], bf16,
                        kind="ExternalInput")
    PH = nc.dram_tensor("ph", [128, off], bf16, kind="ExternalInput")
    WT = nc.dram_tensor("w", [128, KCH * D], bf16, kind="ExternalInput")
    BT = nc.dram_tensor("bias", [D, 1], f32, kind="ExternalInput")
    ONES = nc.dram_tensor("ones", [128, NGROUPS * B], bf16,
                          kind="ExternalInput")
    OUT = nc.dram_tensor("scores", [B, LOCAL_P], f32, kind="ExternalOutput")

    Square = mybir.ActivationFunctionType.Square
    ADD = mybir.AluOpType.add
    MUL = mybir.AluOpType.mult
    MAXOP = mybir.AluOpType.max
    AX = mybir.AxisListType.X

    with TileContext(nc) as tc:
        with (
            tc.tile_pool(name="consts", bufs=1) as consts,
            tc.tile_pool(name="hidp", bufs=6) as hidp,
            tc.tile_pool(name="xbuf", bufs=1) as xbuf,
            tc.tile_pool(name="stats", bufs=1) as stats,
            tc.tile_pool(name="sqp", bufs=3) as sqp,
            tc.tile_pool(name="rnp", bufs=2) as rnp,
            tc.tile_pool(name="mxp", bufs=NGROUPS) as mxp,
            tc.tile_pool(name="strip", bufs=4) as stripp,
            tc.tile_pool(name="l1p", bufs=4) as l1p,
            tc.tile_pool(name="ps_proj", bufs=3, space="PSUM") as ps_proj,
            tc.tile_pool(name="ps_sim", bufs=2, space="PSUM") as ps_sim,
            tc.tile_pool(name="ps_out", bufs=1, space="PSUM") as ps_out,
        ):
            w_t = consts.tile([128, KCH, D], bf16, tag="w")
            nc.sync.dma_start(
                out=w_t[:], in_=WT[:].rearrange("p (k d) -> p k d", d=D)
            )
            b_t = consts.tile([D, 1], f32, tag="b")
            nc.sync.dma_start(out=b_t[:], in_=BT[:])

            xqn = xbuf.tile([128, QCOLS], bf16, tag="xqn")
            xpn = xbuf.tile([128, pmain], bf16, tag="xpn")
            ssq = stats.tile([128, B], f32, tag="ssq")
            ssp = stats.tile([128, LOCAL_P], f32, tag="ssp")
            ssc = stats.tile([128, LOCAL_P], f32, tag="ssc")
            sst = stats.tile([128, LOCAL_P], f32, tag="sst")
            rq = stats.tile([128, B], f32, tag="rq")
            rp = stats.tile([128, LOCAL_P], f32, tag="rp")

            def project(src_ap, ncols):
                """DMA a [128, KCH*ncols] flat slice, contract to PSUM."""
                hid = hidp.tile([128, KCH, 512], bf16, tag="hid")
                hid_v = hid[:, :, :ncols]
                nc.sync.dma_start(
                    out=hid_v, in_=src_ap.rearrange("p (k n) -> p k n", k=KCH)
                )
                ps = ps_proj.tile([128, 512], f32, tag="proj")
                ps_v = ps[:, :ncols]
                for k in range(KCH):
                    nc.tensor.matmul(
                        ps_v, w_t[:, k, :], hid_v[:, k, :],
                        start=(k == 0), stop=(k == KCH - 1),
                    )
                return ps_v

            def sumsq(ps_v, nb, seg, ssdst):
                """ssdst[:, :nb] = per-batch sum of (x+b)^2 from PSUM."""
                sq = sqp.tile([128, 512], bf16, tag="sq")
                sq_v = sq[:, :nb * seg]
                nc.scalar.activation(sq_v, ps_v, Square, bias=b_t[:, 0:1])
                nc.vector.reduce_sum(
                    ssdst, sq_v.rearrange("p (b s) -> p b s", s=seg), axis=AX,
                )

            def rsqrt(ss_ap, n, dst_ap, tagp):
                rt = rnp.tile([128, 16], f32, tag=tagp)
                nc.scalar.sqrt(rt[:, :n], ss_ap)
                nc.vector.reciprocal(dst_ap, rt[:, :n])

            def normalize(ps_v, nb, seg, r_ap, out_ap):
                """out = (x + b) * r, bf16, one vector STT from PSUM."""
                nc.vector.scalar_tensor_tensor(
                    out=out_ap.rearrange("p (b s) -> p b s", s=seg),
                    in0=ps_v.rearrange("p (b s) -> p b s", s=seg),
                    scalar=b_t[:, 0:1],
                    in1=r_ap.to_broadcast([128, nb, seg]),
                    op0=ADD, op1=MUL,
                )

            def q_chunk(c):
                ps_v = project(QH[c], QCHUNK)
                sumsq(ps_v, 12, NQ, ssq[:, c * 12:(c + 1) * 12])
                rsqrt(ssq[:, c * 12:(c + 1) * 12], 12,
                      rq[:, c * 12:(c + 1) * 12], "rq")
                normalize(ps_v, 12, NQ, rq[:, c * 12:(c + 1) * 12],
                          xqn[:, c * QCHUNK:(c + 1) * QCHUNK])

            def corr_chunk(c):
                doff, ncols = c_offs[c]
                ps_v = project(PH[:, doff:doff + KCH * ncols], ncols)
                sq = sqp.tile([128, 512], bf16, tag="sq")
                sq_v = sq[:, :ncols]
                nc.scalar.activation(sq_v, ps_v, Square, bias=b_t[:, 0:1])
                nc.vector.reduce_sum(
                    ssc[:, c * CORR_B:(c + 1) * CORR_B],
                    sq_v.rearrange("p (b s) -> p b s", s=imax), axis=AX,
                )

            def p_half(t, h):
                """Project + normalize half-tile (3 batches) of tile t.
                Scalar evacuates x=(proj+b) to SBUF bf16; GpSimd squares it
                and applies the per-batch 1/norm scale, keeping Vector's
                share to one segmented reduce_sum."""
                j0 = t * BPT + 3 * h
                w = tile_w[t]
                doff, xoff, ncols = p_offs[2 * t + h]
                ps_v = project(PH[:, doff:doff + KCH * ncols], ncols)
                xps = sqp.tile([128, 512], bf16, tag="xps")
                xps_v = xps[:, :ncols]
                nc.scalar.activation(
                    xps_v, ps_v, mybir.ActivationFunctionType.Identity,
                    bias=b_t[:, 0:1],
                )
                sq = sqp.tile([128, 512], bf16, tag="sq")
                sq_v = sq[:, :ncols]
                nc.gpsimd.tensor_tensor(out=sq_v, in0=xps_v, in1=xps_v,
                                        op=MUL)
                nc.vector.reduce_sum(
                    ssp[:, j0:j0 + 3],
                    sq_v.rearrange("p (b s) -> p b s", s=w), axis=AX,
                )
                nc.vector.tensor_tensor(
                    out=sst[:, j0:j0 + 3], in0=ssp[:, j0:j0 + 3],
                    in1=ssc[:, j0:j0 + 3], op=ADD,
                )
                rsqrt(sst[:, j0:j0 + 3], 3, rp[:, j0:j0 + 3], "rp")
                for bi in range(3):
                    nc.gpsimd.tensor_scalar_mul(
                        xpn[:, xoff + bi * w:xoff + (bi + 1) * w],
                        xps_v[:, bi * w:(bi + 1) * w],
                        rp[:, j0 + bi:j0 + bi + 1],
                    )

            # ---- interaction machinery ------------------------------------
            mx_tiles = {}
            strips = {}
            next_t = [0] * NGROUPS
            nsum_emitted = [0]
            # 3 independent accumulator regions in one PSUM bank shorten the
            # serial accumulate chain of the 27 ones-matmuls
            # start=True on the first matmul only: PSUM "start" clears the
            # has_written state of the whole bank, so a later region's start
            # would wipe the other regions' partial sums
            NACC = 3
            score = ps_out.tile([B, NACC * LOCAL_P], f32, tag="score")
            w = tile_w[0]                  # uniform tile width
            assert all(x == w for x in tile_w)

            def direct0(g):
                return g < N_DIRECT

            def emit_pair(g, t):
                rows = min(128, QCOLS - g * 128)
                lhs = xqn[:, g * 128:g * 128 + rows]
                nseg = (NTILES - 1) * BPT if direct0(g) else NTILES * BPT
                if g not in mx_tiles:
                    mx_tiles[g] = mxp.tile([128, LOCAL_P], bf16, tag="mx",
                                           name=f"mx{g}")
                mx = mx_tiles[g]
                sim = ps_sim.tile([128, 2 * 512], f32, tag="sim")
                for h in range(2):
                    xoff = p_offs[2 * t + h][1]
                    nc.tensor.matmul(
                        sim[:rows, h * 512:h * 512 + 3 * w], lhs,
                        xpn[:, xoff:xoff + 3 * w], start=True, stop=True,
                    )
                sim4 = sim[:rows].rearrange("p (u q) -> p u q", q=512)[
                    :, :, :3 * w].rearrange("p u (b w) -> p u b w", w=w)
                if t == 0 and direct0(g):
                    mx6 = mx[:rows, 0:BPT].rearrange("p (u b) -> p u b", u=2)
                    nc.vector.reduce_max(mx6, sim4, axis=AX)
                else:
                    if g not in strips:
                        tag = "s18" if direct0(g) else "s24"
                        strips[g] = stripp.tile([128, nseg * w], bf16,
                                                tag=tag, name=f"strip{g}")
                    strip = strips[g]
                    o = (t - 1) * BPT * w if direct0(g) else t * BPT * w
                    s_v = strip[:rows, o:o + BPT * w].rearrange(
                        "p (u b w) -> p u b w", u=2, b=3)
                    nc.scalar.copy(s_v, sim4)
                if t == NTILES - 1:
                    # merged max tree over the strip tiles
                    strip = strips.pop(g)
                    h2, h4, h8 = w // 2, w // 4, w // 8
                    s3 = strip[:rows].rearrange("p (s w) -> p s w", w=w)
                    l1 = l1p.tile([128, NTILES * BPT * h2], bf16, tag="l1")
                    l1_v = l1[:rows, :nseg * h2].rearrange(
                        "p (s w) -> p s w", w=h2)
                    nc.vector.tensor_tensor(
                        out=l1_v, in0=s3[:, :, :h2], in1=s3[:, :, h2:],
                        op=MAXOP)
                    l2 = l1p.tile([128, NTILES * BPT * h4], bf16, tag="l2")
                    l2_v = l2[:rows, :nseg * h4].rearrange(
                        "p (s w) -> p s w", w=h4)
                    nc.vector.tensor_tensor(
                        out=l2_v, in0=l1_v[:, :, :h4], in1=l1_v[:, :, h4:],
                        op=MAXOP)
                    l3 = l1p.tile([128, NTILES * BPT * h8], bf16, tag="l3")
                    l3_v = l3[:rows, :nseg * h8].rearrange(
                        "p (s w) -> p s w", w=h8)
                    nc.vector.tensor_tensor(
                        out=l3_v, in0=l2_v[:, :, :h8], in1=l2_v[:, :, h8:],
                        op=MAXOP)
                    mo = LOCAL_P - nseg
                    nc.vector.reduce_max(mx[:rows, mo:], l3_v, axis=AX)
                    k = nsum_emitted[0]
                    a = k % NACC
                    nc.tensor.matmul(
                        score[:, a * LOCAL_P:(a + 1) * LOCAL_P],
                        ones_t[:rows, g, :], mx[:rows, :],
                        start=(k == 0), stop=(k == NGROUPS - 1),
                        skip_group_check=True,
                    )
                    nsum_emitted[0] += 1

            def flush_direct(q_cols_done):
                """Emit direct (t=0) interactions for ready direct groups."""
                for g in range(NGROUPS):
                    rows = min(128, QCOLS - g * 128)
                    if g * 128 + rows > q_cols_done:
                        break
                    if next_t[g] == 0 and direct0(g):
                        emit_pair(g, 0)
                        next_t[g] = 1

            def drain_groups(q_cols_done):
                """Run every still-pending tile of covered groups, per group
                consecutively (strip lifetime stays within one group; the
                stationary xqn block is reused across its 8 matmuls)."""
                for g in range(NGROUPS):
                    rows = min(128, QCOLS - g * 128)
                    if g * 128 + rows > q_cols_done:
                        break
                    for t in range(next_t[g], NTILES):
                        emit_pair(g, t)
                        next_t[g] = t + 1

            # ---- schedule -------------------------------------------------
            # q0/q1 first (earliest interactions need them), then correction
            # chunk 0 + passage tile 0 -> first direct reduces start early
            q_chunk(0)
            q_chunk(1)
            corr_chunk(0)
            p_half(0, 0)
            p_half(0, 1)
            flush_direct(2 * QCHUNK)
            corr_chunk(1)
            corr_chunk(2)
            ones_t = consts.tile([128, NGROUPS, B], bf16, tag="ones")
            nc.sync.dma_start(
                out=ones_t[:],
                in_=ONES[:].rearrange("p (g q) -> p g q", q=B),
            )
            for t in range(1, NTILES):
                p_half(t, 0)
                p_half(t, 1)
            for c in range(2, NQCH):
                q_chunk(c)
                drain_groups((c + 1) * QCHUNK)
            drain_groups(QCOLS)

            out_sb = stats.tile([B, LOCAL_P], f32, tag="outsb")
            nc.scalar.copy(out_sb[:], score[:, 0:LOCAL_P])
            for a in range(1, NACC):
                nc.vector.tensor_tensor(
                    out=out_sb[:], in0=out_sb[:],
                    in1=score[:, a * LOCAL_P:(a + 1) * LOCAL_P], op=ADD,
                )
            nc.sync.dma_start(out=OUT[:], in_=out_sb[:])

    nc.compile()
    return nc


def _bf16(a):
    import ml_dtypes
    return np.asarray(a, dtype=np.float32).astype(ml_dtypes.bfloat16)


def _prepare(q_hidden, pos_hidden, neg_hidden, W, b, pos_mask, neg_mask):
    """Shard + pack inputs on host. Returns (in_maps, orders, tile_w, imax)."""
    import ml_dtypes

    Wb = _bf16(W).astype(np.float32)       # the matrix the device will use
    bf = np.asarray(b, dtype=np.float32)
    # pad hidden vector: W^T h* = -b so pad columns project to exactly 0
    hstar, *_ = np.linalg.lstsq(Wb.T.astype(np.float64),
                                -bf.astype(np.float64), rcond=None)
    hstar = _bf16(hstar).astype(np.float32)

    def chunk_cols(hT):
        # [H, n] -> flat [128, KCH * n] so each chunk DMA is contiguous per
        # partition: partition p holds [k, n] row-major
        n = hT.shape[1]
        v = hT.reshape(KCH, 128, n)
        return v.transpose(1, 0, 2).reshape(128, KCH * n)

    qhT = np.ascontiguousarray(
        np.asarray(q_hidden, np.float32).transpose(2, 0, 1).reshape(H, QCOLS))
    qh_c = np.empty((NQCH, 128, KCH * QCHUNK), dtype=ml_dtypes.bfloat16)
    for c in range(NQCH):
        sl = qhT[:, c * QCHUNK:(c + 1) * QCHUNK]
        qh_c[c] = _bf16(chunk_cols(sl))

    Wc = _bf16(
        np.asarray(W, np.float32).reshape(KCH, 128, D)
        .transpose(1, 0, 2).reshape(128, KCH * D)
    )
    bc = np.ascontiguousarray(bf).reshape(D, 1)

    ones = np.zeros((128, NGROUPS * B), dtype=ml_dtypes.bfloat16)
    for g in range(NGROUPS):
        rows = min(128, QCOLS - g * 128)
        for r in range(rows):
            qb = (g * 128 + r) // NQ
            ones[r, g * B + qb] = 1.0

    # per-core valid counts and sort order
    Vs, orders, h_locs, m_locs = [], [], [], []
    for i in range(NCORES):
        sl = slice(i * PB, (i + 1) * PB)
        h_loc = np.concatenate([pos_hidden[sl], neg_hidden[sl]], axis=0)
        m_loc = np.concatenate([pos_mask[sl], neg_mask[sl]], axis=0)
        V = m_loc.sum(axis=1).astype(np.int64)
        order = np.argsort(-V, kind="stable")
        Vs.append(V[order])
        orders.append(order)
        h_locs.append(h_loc)
        m_locs.append(m_loc)
    Vs = np.stack(Vs)                      # [NCORES, 24] sorted desc

    def ceil8(x):
        return (int(x) + 7) // 8 * 8

    # uniform width: the merged tree wants one segment size across tiles
    tile_w = [ceil8(Vs[:, 0].max())] * NTILES
    imax = ceil8(LP - Vs.min())
    assert all(3 * w <= 512 for w in tile_w), tile_w
    assert CORR_B * imax <= 512, imax

    in_maps = []
    for i in range(NCORES):
        order, h_loc, m_loc = orders[i], h_locs[i], m_locs[i]
        pmain_cols = 6 * sum(tile_w)
        main = np.empty((H, pmain_cols), dtype=np.float32)
        corr = np.empty((H, LOCAL_P * imax), dtype=np.float32)
        xoff = 0
        for j, lb in enumerate(order):
            w = tile_w[j // BPT]
            vi = np.flatnonzero(m_loc[lb])
            ii = np.flatnonzero(~m_loc[lb])
            hT = h_loc[lb].T                       # [H, LP]
            blk = np.empty((H, w), dtype=np.float32)
            blk[:, :len(vi)] = hT[:, vi]
            blk[:, len(vi):] = hstar[:, None]
            main[:, xoff:xoff + w] = blk
            xoff += w
            cb = np.empty((H, imax), dtype=np.float32)
            cb[:, :len(ii)] = hT[:, ii]
            cb[:, len(ii):] = hstar[:, None]
            corr[:, j * imax:(j + 1) * imax] = cb
        # flat ph layout: 8 half-tile chunks then 3 correction chunks
        segs = []
        xoff = 0
        for t in range(NTILES):
            for h in range(2):
                n = 3 * tile_w[t]
                segs.append(chunk_cols(main[:, xoff:xoff + n]))
                xoff += n
        for c in range(NCORR):
            n = CORR_B * imax
            segs.append(chunk_cols(corr[:, c * n:(c + 1) * n]))
        ph = _bf16(np.concatenate(segs, axis=1))
        in_maps.append({
            "qh": qh_c, "ph": ph, "w": Wc, "bias": bc, "ones": ones,
        })
    return in_maps, orders, tile_w, imax


def _assemble(results, orders):
    out = np.zeros((B, 2 * B), dtype=np.float32)
    for i in range(NCORES):
        sc = results[i]["scores"]                  # [96, 24]
        for j, lb in enumerate(orders[i]):
            if lb < PB:
                out[:, i * PB + lb] = sc[:, j]
            else:
                out[:, B + i * PB + (lb - PB)] = sc[:, j]
    return out


def _run(inputs, trace=False):
    from concourse.bass_utils import run_bass_kernel_spmd

    in_maps, orders, tile_w, imax = _prepare(**inputs)
    nc = _build(tuple(tile_w), imax)
    res = run_bass_kernel_spmd(nc, in_maps, list(range(NCORES)), trace=trace)
    return _assemble(res.results, orders), res


def kernel(**inputs) -> np.ndarray:
    out, _ = _run(inputs, trace=False)
    return out


def kernel_profiled(**inputs):
    out, res = _run(inputs, trace=True)
    return out, res


# revision 21
# speedup vs baseline: 1.3869x; 1.3869x over previous
"""ColBERT late-interaction kernel for 8 Trainium2 NeuronCores (v2).

Math (per reference):
  x = h @ W + b                      (projection, H=768 -> D=128)
  v = x / ||x||_2(seq axis)          (normalize over the SEQUENCE axis,
                                      norm includes masked tokens)
  sim[q,p,n,l] = <q_v[q,n], p_v[p,l]>  (masked tokens excluded from max)
  scores[q,p] = sum_n max_{l valid} sim[q,p,n,l]
  out = concat(pos_scores, neg_scores, axis=1)   # [96, 192]

Sharding: passage-parallel. Every core projects ALL queries and a 1/8 shard
of pos+neg passages (12+12 batches), computes the full-query x local-passage
score block [96, 24]; the host stitches columns.

v2 design notes:
  - Hidden tensors ship as bf16 (halves HBM traffic); projections contract
    bf16 x bf16 with fp32 PSUM accumulate.
  - No mask tensor at all: the host solves W^T h* = -b (on the bf16-rounded
    W) and substitutes h* for pad slots, so those columns project to ~0 and
    drop out of both the max (true max > 0) and the norm. Invalid tokens are
    moved to a compact "correction" block so the sequence-axis sum-of-squares
    still includes them, exactly as the reference does.
  - Passage batches are sorted by valid count; each tile of 6 batches is
    compacted to W_t columns (tile max valid count, rounded up to 8).
  - Sum-of-squares: one ACT Square per chunk (bias folded) to SBUF, then a
    segmented vector reduce_sum -- avoids the per-batch ACT-accumulate
    instruction-overhead wall.
  - Normalization: one scalar_tensor_tensor per chunk on Vector reads the
    projection PSUM directly: out = (x + b) * rsqrt(ss), bf16 out.
  - MaxSim drain is split across engines. PSUM can only be read by Vector
    (0.96 elem/ns/lane) and Scalar (1.2 elem/ns/lane), and Vector's reduce
    is locked at 1x. Role 'V' blocks: direct vector reduce_max from PSUM.
    Role 'G' blocks: Scalar ACT-copies the sim block to SBUF as bf16,
    GpSimd does the first max-halving level, Vector finishes with 2x-mode
    bf16 tensor_tensor max levels + a short reduce.
  - The sum-over-n runs as a ones-matmul per row-group that ACCUMULATES into
    a single PSUM bank across all 27 groups (start only on the first), so the
    epilogue is one copy + one DMA.
"""

import numpy as np

B, NQ, LP, H, D = 96, 35, 180, 768, 128
NCORES = 8
PB = B // NCORES          # 12 passage batches per core per side
LOCAL_P = 2 * PB          # 24 local passage batches (pos then neg)
QCOLS = B * NQ            # 3360 query columns
KCH = H // 128            # 6 contraction chunks
QCHUNK = 420              # 12 query batches per projection chunk
NQCH = QCOLS // QCHUNK    # 8
NGROUPS = (QCOLS + 127) // 128       # 27 interaction row-groups
BPT = 6                   # passage batches per tile
NTILES = LOCAL_P // BPT   # 4
NCORR = 3                 # correction chunks
CORR_B = LOCAL_P // NCORR            # 8 batches per correction chunk


# Groups g < N_DIRECT drain tile 0 by a direct vector reduce_max from PSUM
# and tiles 1-3 through the scalar-copy + vector TT-max tree; groups
# g >= N_DIRECT push all 4 tiles through the tree (cheaper for Vector,
# pricier for Scalar) — the knob balances the two engines.
N_DIRECT = 16


def _build(tile_w, imax):
    import concourse.bacc as bacc
    from concourse import mybir
    from concourse.tile import TileContext

    f32 = mybir.dt.float32
    bf16 = mybir.dt.bfloat16

    tile_w = list(tile_w)
    # half-tile layout: per tile, 2 PSUM banks x 3 batches x W columns
    assert all(3 * w <= 512 for w in tile_w)
    pmain = 6 * sum(tile_w)               # compacted passage columns
    pcorr = LOCAL_P * imax                # correction columns
    # flat per-partition layouts (chunk-major inside each chunk)
    p_offs = []                           # (dram_off, xpn_off, ncols) per half-tile
    off = 0
    xoff = 0
    for t in range(NTILES):
        for h in range(2):
            p_offs.append((off, xoff, 3 * tile_w[t]))
            off += KCH * 3 * tile_w[t]
            xoff += 3 * tile_w[t]
    c_offs = []
    for c in range(NCORR):
        c_offs.append((off, CORR_B * imax))
        off += KCH * CORR_B * imax

    nc = bacc.Bacc(target_bir_lowering=False)

    QH = nc.dram_tensor("qh", [NQCH, 128, KCH * QCHUNK], bf16,
                        kind="ExternalInput")
    PH = nc.dram_tensor("ph", [128, off], bf16, kind="ExternalInput")
    WT = nc.dram_tensor("w", [128, KCH * D], bf16, kind="ExternalInput")
    BT = nc.dram_tensor("bias", [D, 1], f32, kind="ExternalInput")
    ONES = nc.dram_tensor("ones", [128, NGROUPS * B], bf16,
                          kind="ExternalInput")
    OUT = nc.dram_tensor("scores", [B, LOCAL_P], f32, kind="ExternalOutput")

    Square = mybir.ActivationFunctionType.Square
    ADD = mybir.AluOpType.add
    MUL = mybir.AluOpType.mult
    MAXOP = mybir.AluOpType.max
    AX = mybir.AxisListType.X

    with TileContext(nc) as tc:
        with (
            tc.tile_pool(name="consts", bufs=1) as consts,
            tc.tile_pool(name="hidp", bufs=6) as hidp,
            tc.tile_pool(name="xbuf", bufs=1) as xbuf,
            tc.tile_pool(name="stats", bufs=1) as stats,
            tc.tile_pool(name="sqp", bufs=3) as sqp,
            tc.tile_pool(name="rnp", bufs=2) as rnp,
            tc.tile_pool(name="mxp", bufs=NGROUPS) as mxp,
            tc.tile_pool(name="strip", bufs=4) as stripp,
            tc.tile_pool(name="l1p", bufs=4) as l1p,
            tc.tile_pool(name="ps_proj", bufs=3, space="PSUM") as ps_proj,
            tc.tile_pool(name="ps_sim", bufs=2, space="PSUM") as ps_sim,
            tc.tile_pool(name="ps_out", bufs=1, space="PSUM") as ps_out,
        ):
            w_t = consts.tile([128, KCH, D], bf16, tag="w")
            nc.sync.dma_start(
                out=w_t[:], in_=WT[:].rearrange("p (k d) -> p k d", d=D)
            )
            b_t = consts.tile([D, 1], f32, tag="b")
            nc.sync.dma_start(out=b_t[:], in_=BT[:])

            xqn = xbuf.tile([128, QCOLS], bf16, tag="xqn")
            xpn = xbuf.tile([128, pmain], bf16, tag="xpn")
            ssq = stats.tile([128, B], f32, tag="ssq")
            ssp = stats.tile([128, LOCAL_P], f32, tag="ssp")
            ssc = stats.tile([128, LOCAL_P], f32, tag="ssc")
            sst = stats.tile([128, LOCAL_P], f32, tag="sst")
            rq = stats.tile([128, B], f32, tag="rq")
            rp = stats.tile([128, LOCAL_P], f32, tag="rp")

            def project(src_ap, ncols):
                """DMA a [128, KCH*ncols] flat slice, contract to PSUM."""
                hid = hidp.tile([128, KCH, 512], bf16, tag="hid")
                hid_v = hid[:, :, :ncols]
                nc.sync.dma_start(
                    out=hid_v, in_=src_ap.rearrange("p (k n) -> p k n", k=KCH)
                )
                ps = ps_proj.tile([128, 512], f32, tag="proj")
                ps_v = ps[:, :ncols]
                for k in range(KCH):
                    nc.tensor.matmul(
                        ps_v, w_t[:, k, :], hid_v[:, k, :],
                        start=(k == 0), stop=(k == KCH - 1),
                    )
                return ps_v

            def sumsq(ps_v, nb, seg, ssdst):
                """ssdst[:, :nb] = per-batch sum of (x+b)^2 from PSUM."""
                sq = sqp.tile([128, 512], bf16, tag="sq")
                sq_v = sq[:, :nb * seg]
                nc.scalar.activation(sq_v, ps_v, Square, bias=b_t[:, 0:1])
                nc.vector.reduce_sum(
                    ssdst, sq_v.rearrange("p (b s) -> p b s", s=seg), axis=AX,
                )

            def rsqrt(ss_ap, n, dst_ap, tagp):
                rt = rnp.tile([128, 16], f32, tag=tagp)
                nc.scalar.sqrt(rt[:, :n], ss_ap)
                nc.vector.reciprocal(dst_ap, rt[:, :n])

            def normalize(ps_v, nb, seg, r_ap, out_ap):
                """out = (x + b) * r, bf16, one vector STT from PSUM."""
                nc.vector.scalar_tensor_tensor(
                    out=out_ap.rearrange("p (b s) -> p b s", s=seg),
                    in0=ps_v.rearrange("p (b s) -> p b s", s=seg),
                    scalar=b_t[:, 0:1],
                    in1=r_ap.to_broadcast([128, nb, seg]),
                    op0=ADD, op1=MUL,
                )

            def q_chunk(c):
                ps_v = project(QH[c], QCHUNK)
                sumsq(ps_v, 12, NQ, ssq[:, c * 12:(c + 1) * 12])
                rsqrt(ssq[:, c * 12:(c + 1) * 12], 12,
                      rq[:, c * 12:(c + 1) * 12], "rq")
                normalize(ps_v, 12, NQ, rq[:, c * 12:(c + 1) * 12],
                          xqn[:, c * QCHUNK:(c + 1) * QCHUNK])

            def corr_chunk(c):
                doff, ncols = c_offs[c]
                ps_v = project(PH[:, doff:doff + KCH * ncols], ncols)
                sq = sqp.tile([128, 512], bf16, tag="sq")
                sq_v = sq[:, :ncols]
                nc.scalar.activation(sq_v, ps_v, Square, bias=b_t[:, 0:1])
                nc.vector.reduce_sum(
                    ssc[:, c * CORR_B:(c + 1) * CORR_B],
                    sq_v.rearrange("p (b s) -> p b s", s=imax), axis=AX,
                )

            def p_half(t, h):
                """Project + normalize half-tile (3 batches) of tile t.
                Scalar evacuates x=(proj+b) to SBUF bf16; GpSimd squares it
                and applies the per-batch 1/norm scale, keeping Vector's
                share to one segmented reduce_sum."""
                j0 = t * BPT + 3 * h
                w = tile_w[t]
                doff, xoff, ncols = p_offs[2 * t + h]
                ps_v = project(PH[:, doff:doff + KCH * ncols], ncols)
                sumsq(ps_v, 3, w, ssp[:, j0:j0 + 3])
                nc.vector.tensor_tensor(
                    out=sst[:, j0:j0 + 3], in0=ssp[:, j0:j0 + 3],
                    in1=ssc[:, j0:j0 + 3], op=ADD,
                )
                rsqrt(sst[:, j0:j0 + 3], 3, rp[:, j0:j0 + 3], "rp")
                normalize(ps_v, 3, w, rp[:, j0:j0 + 3],
                          xpn[:, xoff:xoff + ncols])

            # ---- interaction machinery ------------------------------------
            mx_tiles = {}
            strips = {}
            next_t = [0] * NGROUPS
            nsum_emitted = [0]
            # 3 independent accumulator regions in one PSUM bank shorten the
            # serial accumulate chain of the 27 ones-matmuls
            # start=True on the first matmul only: PSUM "start" clears the
            # has_written state of the whole bank, so a later region's start
            # would wipe the other regions' partial sums
            NACC = 3
            score = ps_out.tile([B, NACC * LOCAL_P], f32, tag="score")
            w = tile_w[0]                  # uniform tile width
            assert all(x == w for x in tile_w)

            def direct0(g):
                return g < N_DIRECT

            def emit_pair(g, t):
                rows = min(128, QCOLS - g * 128)
                lhs = xqn[:, g * 128:g * 128 + rows]
                nseg = (NTILES - 1) * BPT if direct0(g) else NTILES * BPT
                if g not in mx_tiles:
                    mx_tiles[g] = mxp.tile([128, LOCAL_P], bf16, tag="mx",
                                           name=f"mx{g}")
                mx = mx_tiles[g]
                sim = ps_sim.tile([128, 2 * 512], f32, tag="sim")
                for h in range(2):
                    xoff = p_offs[2 * t + h][1]
                    nc.tensor.matmul(
                        sim[:rows, h * 512:h * 512 + 3 * w], lhs,
                        xpn[:, xoff:xoff + 3 * w], start=True, stop=True,
                    )
                sim4 = sim[:rows].rearrange("p (u q) -> p u q", q=512)[
                    :, :, :3 * w].rearrange("p u (b w) -> p u b w", w=w)
                if t == 0 and direct0(g):
                    mx6 = mx[:rows, 0:BPT].rearrange("p (u b) -> p u b", u=2)
                    nc.vector.reduce_max(mx6, sim4, axis=AX)
                else:
                    if g not in strips:
                        tag = "s18" if direct0(g) else "s24"
                        strips[g] = stripp.tile([128, nseg * w], bf16,
                                                tag=tag, name=f"strip{g}")
                    strip = strips[g]
                    o = (t - 1) * BPT * w if direct0(g) else t * BPT * w
                    s_v = strip[:rows, o:o + BPT * w].rearrange(
                        "p (u b w) -> p u b w", u=2, b=3)
                    nc.scalar.copy(s_v, sim4)
                if t == NTILES - 1:
                    # merged max tree over the strip tiles
                    strip = strips.pop(g)
                    h2, h4, h8 = w // 2, w // 4, w // 8
                    s3 = strip[:rows].rearrange("p (s w) -> p s w", w=w)
                    l1 = l1p.tile([128, NTILES * BPT * h2], bf16, tag="l1")
                    l1_v = l1[:rows, :nseg * h2].rearrange(
                        "p (s w) -> p s w", w=h2)
                    nc.vector.tensor_tensor(
                        out=l1_v, in0=s3[:, :, :h2], in1=s3[:, :, h2:],
                        op=MAXOP)
                    l2 = l1p.tile([128, NTILES * BPT * h4], bf16, tag="l2")
                    l2_v = l2[:rows, :nseg * h4].rearrange(
                        "p (s w) -> p s w", w=h4)
                    nc.vector.tensor_tensor(
                        out=l2_v, in0=l1_v[:, :, :h4], in1=l1_v[:, :, h4:],
                        op=MAXOP)
                    l3 = l1p.tile([128, NTILES * BPT * h8], bf16, tag="l3")
                    l3_v = l3[:rows, :nseg * h8].rearrange(
                        "p (s w) -> p s w", w=h8)
                    nc.vector.tensor_tensor(
                        out=l3_v, in0=l2_v[:, :, :h8], in1=l2_v[:, :, h8:],
                        op=MAXOP)
                    mo = LOCAL_P - nseg
                    nc.vector.reduce_max(mx[:rows, mo:], l3_v, axis=AX)
                    k = nsum_emitted[0]
                    a = k % NACC
                    nc.tensor.matmul(
                        score[:, a * LOCAL_P:(a + 1) * LOCAL_P],
                        ones_t[:rows, g, :], mx[:rows, :],
                        start=(k == 0), stop=(k == NGROUPS - 1),
                        skip_group_check=True,
                    )
                    nsum_emitted[0] += 1

            def flush_direct(q_cols_done):
                """Emit direct (t=0) interactions for ready direct groups."""
                for g in range(NGROUPS):
                    rows = min(128, QCOLS - g * 128)
                    if g * 128 + rows > q_cols_done:
                        break
                    if next_t[g] == 0 and direct0(g):
                        emit_pair(g, 0)
                        next_t[g] = 1

            def drain_groups(q_cols_done):
                """Run every still-pending tile of covered groups, per group
                consecutively (strip lifetime stays within one group; the
                stationary xqn block is reused across its 8 matmuls)."""
                for g in range(NGROUPS):
                    rows = min(128, QCOLS - g * 128)
                    if g * 128 + rows > q_cols_done:
                        break
                    for t in range(next_t[g], NTILES):
                        emit_pair(g, t)
                        next_t[g] = t + 1

            # ---- schedule -------------------------------------------------
            # q0/q1 first (earliest interactions need them), then correction
            # chunk 0 + passage tile 0 -> first direct reduces start early
            q_chunk(0)
            q_chunk(1)
            corr_chunk(0)
            p_half(0, 0)
            p_half(0, 1)
            flush_direct(2 * QCHUNK)
            corr_chunk(1)
            corr_chunk(2)
            ones_t = consts.tile([128, NGROUPS, B], bf16, tag="ones")
            nc.sync.dma_start(
                out=ones_t[:],
                in_=ONES[:].rearrange("p (g q) -> p g q", q=B),
            )
            for t in range(1, NTILES):
                p_half(t, 0)
                p_half(t, 1)
            for c in range(2, NQCH):
                q_chunk(c)
                drain_groups((c + 1) * QCHUNK)
            drain_groups(QCOLS)

            out_sb = stats.tile([B, LOCAL_P], f32, tag="outsb")
            nc.scalar.copy(out_sb[:], score[:, 0:LOCAL_P])
            for a in range(1, NACC):
                nc.vector.tensor_tensor(
                    out=out_sb[:], in0=out_sb[:],
                    in1=score[:, a * LOCAL_P:(a + 1) * LOCAL_P], op=ADD,
                )
            nc.sync.dma_start(out=OUT[:], in_=out_sb[:])

    nc.compile()
    return nc


def _bf16(a):
    import ml_dtypes
    return np.asarray(a, dtype=np.float32).astype(ml_dtypes.bfloat16)


def _prepare(q_hidden, pos_hidden, neg_hidden, W, b, pos_mask, neg_mask):
    """Shard + pack inputs on host. Returns (in_maps, orders, tile_w, imax)."""
    import ml_dtypes

    Wb = _bf16(W).astype(np.float32)       # the matrix the device will use
    bf = np.asarray(b, dtype=np.float32)
    # pad hidden vector: W^T h* = -b so pad columns project to exactly 0
    hstar, *_ = np.linalg.lstsq(Wb.T.astype(np.float64),
                                -bf.astype(np.float64), rcond=None)
    hstar = _bf16(hstar).astype(np.float32)

    def chunk_cols(hT):
        # [H, n] -> flat [128, KCH * n] so each chunk DMA is contiguous per
        # partition: partition p holds [k, n] row-major
        n = hT.shape[1]
        v = hT.reshape(KCH, 128, n)
        return v.transpose(1, 0, 2).reshape(128, KCH * n)

    qhT = np.ascontiguousarray(
        np.asarray(q_hidden, np.float32).transpose(2, 0, 1).reshape(H, QCOLS))
    qh_c = np.empty((NQCH, 128, KCH * QCHUNK), dtype=ml_dtypes.bfloat16)
    for c in range(NQCH):
        sl = qhT[:, c * QCHUNK:(c + 1) * QCHUNK]
        qh_c[c] = _bf16(chunk_cols(sl))

    Wc = _bf16(
        np.asarray(W, np.float32).reshape(KCH, 128, D)
        .transpose(1, 0, 2).reshape(128, KCH * D)
    )
    bc = np.ascontiguousarray(bf).reshape(D, 1)

    ones = np.zeros((128, NGROUPS * B), dtype=ml_dtypes.bfloat16)
    for g in range(NGROUPS):
        rows = min(128, QCOLS - g * 128)
        for r in range(rows):
            qb = (g * 128 + r) // NQ
            ones[r, g * B + qb] = 1.0

    # per-core valid counts and sort order
    Vs, orders, h_locs, m_locs = [], [], [], []
    for i in range(NCORES):
        sl = slice(i * PB, (i + 1) * PB)
        h_loc = np.concatenate([pos_hidden[sl], neg_hidden[sl]], axis=0)
        m_loc = np.concatenate([pos_mask[sl], neg_mask[sl]], axis=0)
        V = m_loc.sum(axis=1).astype(np.int64)
        order = np.argsort(-V, kind="stable")
        Vs.append(V[order])
        orders.append(order)
        h_locs.append(h_loc)
        m_locs.append(m_loc)
    Vs = np.stack(Vs)                      # [NCORES, 24] sorted desc

    def ceil8(x):
        return (int(x) + 7) // 8 * 8

    # uniform width: the merged tree wants one segment size across tiles
    tile_w = [ceil8(Vs[:, 0].max())] * NTILES
    imax = ceil8(LP - Vs.min())
    assert all(3 * w <= 512 for w in tile_w), tile_w
    assert CORR_B * imax <= 512, imax

    in_maps = []
    for i in range(NCORES):
        order, h_loc, m_loc = orders[i], h_locs[i], m_locs[i]
        pmain_cols = 6 * sum(tile_w)
        main = np.empty((H, pmain_cols), dtype=np.float32)
        corr = np.empty((H, LOCAL_P * imax), dtype=np.float32)
        xoff = 0
        for j, lb in enumerate(order):
            w = tile_w[j // BPT]
            vi = np.flatnonzero(m_loc[lb])
            ii = np.flatnonzero(~m_loc[lb])
            hT = h_loc[lb].T                       # [H, LP]
            blk = np.empty((H, w), dtype=np.float32)
            blk[:, :len(vi)] = hT[:, vi]
            blk[:, len(vi):] = hstar[:, None]
            main[:, xoff:xoff + w] = blk
            xoff += w
            cb = np.empty((H, imax), dtype=np.float32)
            cb[:, :len(ii)] = hT[:, ii]
            cb[:, len(ii):] = hstar[:, None]
            corr[:, j * imax:(j + 1) * imax] = cb
        # flat ph layout: 8 half-tile chunks then 3 correction chunks
        segs = []
        xoff = 0
        for t in range(NTILES):
            for h in range(2):
                n = 3 * tile_w[t]
                segs.append(chunk_cols(main[:, xoff:xoff + n]))
                xoff += n
        for c in range(NCORR):
            n = CORR_B * imax
            segs.append(chunk_cols(corr[:, c * n:(c + 1) * n]))
        ph = _bf16(np.concatenate(segs, axis=1))
        in_maps.append({
            "qh": qh_c, "ph": ph, "w": Wc, "bias": bc, "ones": ones,
        })
    return in_maps, orders, tile_w, imax


def _assemble(results, orders):
    out = np.zeros((B, 2 * B), dtype=np.float32)
    for i in range(NCORES):
        sc = results[i]["scores"]                  # [96, 24]
        for j, lb in enumerate(orders[i]):
            if lb < PB:
                out[:, i * PB + lb] = sc[:, j]
            else:
                out[:, B + i * PB + (lb - PB)] = sc[:, j]
    return out


def _run(inputs, trace=False):
    from concourse.bass_utils import run_bass_kernel_spmd

    in_maps, orders, tile_w, imax = _prepare(**inputs)
    nc = _build(tuple(tile_w), imax)
    res = run_bass_kernel_spmd(nc, in_maps, list(range(NCORES)), trace=trace)
    return _assemble(res.results, orders), res


def kernel(**inputs) -> np.ndarray:
    out, _ = _run(inputs, trace=False)
    return out


def kernel_profiled(**inputs):
    out, res = _run(inputs, trace=True)
    return out, res
